# revision 4
# baseline (speedup 1.0000x reference)
# Multi-head attention (B=4, S=2048, D=512, H=8) on 8 Trainium2 NeuronCores.
#
# Sharding: core c handles batch c//2 and query rows [(c%2)*1024, (c%2+1)*1024)
# for all 8 heads over all 2048 keys. Output slices are disjoint -> no
# collectives needed.
#
# v2 design notes (vs the v1 baseline at 188us):
#   - mask machinery removed entirely: masked/padded keys ship ZERO xk/xv
#     columns and a 0 entry in the "kones" vector that fills the denominator
#     row of the v stationary. exp(q.0)=1 but both its v row and its ones
#     entry are 0, so padded keys contribute nothing to numerator or
#     denominator. Works for the compacted AND the dense fallback layout.
#   - k bias dropped (constant-per-query term, cancels exactly in softmax).
#     v bias + o bias folded on host: bo2 = bo + Wo @ bv (softmax rows sum
#     to 1, so the bv term rides through attention unchanged).
#   - v projection interleaved into the j=0 attention loop so the scalar
#     engine's exp stream (the true bottleneck, ~72 x 1us) starts ~15us
#     earlier and vproj hides under it.
#   - odd heads store v as [1 | v] so their attnv PSUM lands at partitions
#     63..127: the normalize multiply writes outTn[64:128] lane-aligned,
#     killing v1's 4 SBUF->SBUF shuffle DMAs.
#   - reciprocal: v1 spent 6.5us per [1,1024] DVE reciprocal (52us total!).
#     Now the two denominators of a head pair round-trip through DRAM into a
#     [128,16] tile and use one reciprocal_approx_fast (~0.2us).
#   - output projection runs in ONE wave: 8 x [128,512] single-bank PSUM
#     accumulators, jj-outer, so jj=0..2 passes overlap the last pair's
#     normalization; bias is added by the DVE during PSUM evacuation.
#   - PE HAM warmup: ~10 dummy matmuls during the initial DMA wait so the
#     clock gate is at 8/8 when real matmuls arrive; exp table preloaded the
#     same way.
#   - input DMAs spread across scalar/sync/vector/gpsimd rings, output DMAs
#     round-robin over 4 rings.

import sys
import os

for _p in ("/opt/trn_rl_repo", "/root/.axon_site/_ro/trn_rl_repo"):
    if os.path.isdir(_p) and _p not in sys.path:
        sys.path.append(_p)

import numpy as np

B, S, D, H = 4, 2048, 512, 8
DK = D // H          # 64
N_CORES = 8
SQ = S // 2          # 1024 query rows per core
SKC = 1152           # compacted key capacity (9 tiles of 128)

_compiled = {}       # skeys -> Bacc
last_results = None  # BassKernelResults of the most recent run (for test.py)


def _build(skeys):
    import concourse.bass as bass  # noqa: F401
    from concourse import bacc
    import concourse.tile as tile
    import concourse.mybir as mybir

    fp32 = mybir.dt.float32
    bf16 = mybir.dt.bfloat16
    EXP = mybir.ActivationFunctionType.Exp
    nkt = skeys // 128
    # key-side projection chunks of up to 512 columns
    kchunks = []
    off = 0
    while off < skeys:
        w = min(512, skeys - off)
        kchunks.append((off, w))
        off += w

    nc = bacc.Bacc("TRN2", target_bir_lowering=False, debug=False,
                   num_devices=N_CORES)

    xq = nc.dram_tensor("xq", [D, SQ], bf16, kind="ExternalInput")
    xk = nc.dram_tensor("xk", [D, skeys], bf16, kind="ExternalInput")
    xv = nc.dram_tensor("xv", [D, skeys], bf16, kind="ExternalInput")
    wq = nc.dram_tensor("wq", [D, D], bf16, kind="ExternalInput")
    wk = nc.dram_tensor("wk", [D, D], bf16, kind="ExternalInput")
    wv = nc.dram_tensor("wv", [D, D], bf16, kind="ExternalInput")
    wo = nc.dram_tensor("wo", [D, D], bf16, kind="ExternalInput")
    bq = nc.dram_tensor("bq", [128, 4], fp32, kind="ExternalInput")
    ko = nc.dram_tensor("ko", [128, nkt], bf16, kind="ExternalInput")
    bo2 = nc.dram_tensor("bo2", [1, D], fp32, kind="ExternalInput")
    out = nc.dram_tensor("out", [SQ, D], fp32, kind="ExternalOutput")
    rds = nc.dram_tensor("rds", [H, SQ], fp32)   # scratch: denominators
    rds2 = nc.dram_tensor("rds2", [H, SQ], fp32)  # scratch: 1/denominator

    with tile.TileContext(nc) as tc:
        with (
            tc.tile_pool(name="consts", bufs=1) as consts,
            tc.tile_pool(name="xin", bufs=1) as xin,
            tc.tile_pool(name="qk", bufs=1) as qk,
            tc.tile_pool(name="vp", bufs=1) as vp,
            tc.tile_pool(name="stp", bufs=4) as stp,
            tc.tile_pool(name="small", bufs=2) as small,
            tc.tile_pool(name="osb", bufs=3) as osb,
            tc.tile_pool(name="pst", bufs=2, space="PSUM") as pst,
            tc.tile_pool(name="pout", bufs=2, space="PSUM") as pout,
        ):
            # ---- warmup: exp table load + PE HAM un-throttle ----
            warm = consts.tile([128, 512], bf16, tag="warm")
            nc.vector.memset(warm[:], 0.25)
            warm_act = consts.tile([1, 512], bf16, tag="warma")
            nc.scalar.activation(out=warm_act[:], in_=warm[0:1, :], func=EXP,
                                 scale=1.0)
            wps = pst.tile([128, 1024], fp32, tag="st", name="warm_ps")
            for _ in range(10):
                nc.tensor.matmul(wps[:, 0:512], warm[:, 0:128],
                                 warm[:, 0:512], start=True, stop=True)

            # ---- input DMAs, spread across rings ----
            # scalar: wq, wk, wo | sync: xq | vector: xk, xv | gpsimd: rest
            wq_sb = consts.tile([128, 4, D], bf16, tag="wq")
            for kc in range(4):
                nc.scalar.dma_start(out=wq_sb[:, kc, :],
                                    in_=wq[kc * 128:(kc + 1) * 128, :])
            xq_sb = xin.tile([128, 4, SQ], bf16, tag="xq")
            nc.sync.dma_start(
                out=xq_sb[:],
                in_=xq.rearrange("(kc p) s -> p kc s", p=128))
            xk_sb = xin.tile([128, 4, skeys], bf16, tag="xk")
            for off, w in kchunks:
                nc.gpsimd.dma_start(
                    out=xk_sb[:, :, off:off + w],
                    in_=xk[:, off:off + w]
                    .rearrange("(kc p) s -> p kc s", p=128))
            wk_sb = consts.tile([128, 4, D], bf16, tag="wk")
            for kc in range(4):
                nc.scalar.dma_start(out=wk_sb[:, kc, :],
                                    in_=wk[kc * 128:(kc + 1) * 128, :])
            bq_sb = consts.tile([128, 4], fp32, tag="bq")
            nc.gpsimd.dma_start(out=bq_sb[:], in_=bq[:, :])
            ko_sb = consts.tile([128, nkt], bf16, tag="ko")
            nc.gpsimd.dma_start(out=ko_sb[:], in_=ko[:, :])
            wv_sb = consts.tile([128, 4, D], bf16, tag="wv")
            for kc in range(4):
                nc.scalar.dma_start(out=wv_sb[:, kc, :],
                                    in_=wv[kc * 128:(kc + 1) * 128, :])
            xv_sb = xin.tile([128, 4, skeys], bf16, tag="xv")
            for off, w in kchunks:
                nc.sync.dma_start(
                    out=xv_sb[:, :, off:off + w],
                    in_=xv[:, off:off + w]
                    .rearrange("(kc p) s -> p kc s", p=128))
            # WoT rows packed by head pair: [128, 4, 512]
            wo_sb = consts.tile([128, 4, D], bf16, tag="wo")
            nc.scalar.dma_start(out=wo_sb[:],
                                in_=wo.rearrange("(j p) n -> p j n", p=128))
            bo2_sb = consts.tile([128, D], fp32, tag="bo2")
            nc.gpsimd.dma_start(out=bo2_sb[:],
                                in_=bo2[0:1, :].partition_broadcast(128))

            # ---- q projection (head pairs stacked on partitions) ----
            # stationary per (j,kc) streams both query halves -> 16 LDWs
            qT = qk.tile([128, 4, SQ], bf16, tag="qT")
            for j in range(4):
                pqA = pst.tile([128, 1024], fp32, tag="st", name=f"pqA_{j}")
                pqB = pst.tile([128, 1024], fp32, tag="st", name=f"pqB_{j}")
                for kc in range(4):
                    for qc, p in ((0, pqA), (1, pqB)):
                        nc.tensor.matmul(
                            p[:, 0:512],
                            wq_sb[:, kc, j * 128:(j + 1) * 128],
                            xq_sb[:, kc, qc * 512:(qc + 1) * 512],
                            start=(kc == 0), stop=(kc == 3))
                nc.scalar.add(qT[:, j, 0:512], pqA[:, 0:512],
                              bq_sb[:, j:j + 1])
                nc.scalar.add(qT[:, j, 512:1024], pqB[:, 0:512],
                              bq_sb[:, j:j + 1])

            # ---- k projection (no bias: cancels in softmax) ----
            kT = qk.tile([128, 4, skeys], bf16, tag="kT")
            # group the <=512 chunks in pairs per [128,1024] psum tile
            cgroups = [kchunks[i:i + 2] for i in range(0, len(kchunks), 2)]
            for j in range(4):
                for gi, grp in enumerate(cgroups):
                    pk = pst.tile([128, 1024], fp32, tag="st",
                                  name=f"pk_{j}_{gi}")
                    for kc in range(4):
                        for ci, (off, w) in enumerate(grp):
                            nc.tensor.matmul(
                                pk[:, ci * 512:ci * 512 + w],
                                wk_sb[:, kc, j * 128:(j + 1) * 128],
                                xk_sb[:, kc, off:off + w],
                                start=(kc == 0), stop=(kc == 3))
                    for ci, (off, w) in enumerate(grp):
                        nc.scalar.copy(kT[:, j, off:off + w],
                                       pk[:, ci * 512:ci * 512 + w])

            # ---- v stationaries: [v | kones] per head ----
            v_sb = vp.tile([128, nkt, H, DK + 1], bf16, tag="v")
            for h in range(H):
                nc.vector.tensor_copy(out=v_sb[:, :, h, DK], in_=ko_sb[:, :])

            # ---- attention, one head pair at a time ----
            # per sk: scores (A then B), exp on scalar, attnv of sk-1.
            # v projection for key tile sk interleaved into the j=0 loop.
            outTn = qk.tile([128, 4, SQ], bf16, tag="outTn")
            for j in range(4):
                po0 = pout.tile([128, 1024], fp32, tag="po", name=f"po0_{j}")
                po1 = pout.tile([128, 1024], fp32, tag="po", name=f"po1_{j}")
                sts = []
                for sk in range(nkt):
                    if j == 0:
                        pv = pst.tile([128, 1024], fp32, tag="st",
                                      name=f"pv_{sk}")
                        for kc in range(4):
                            nc.tensor.matmul(
                                pv[:, 0:512],
                                xv_sb[:, kc, sk * 128:(sk + 1) * 128],
                                wv_sb[:, kc, :],
                                start=(kc == 0), stop=(kc == 3))
                        nc.vector.tensor_copy(
                            out=v_sb[:, sk, :, 0:DK],
                            in_=pv[:, 0:512].rearrange(
                                "p (h m) -> p h m", h=H))
                    psA = pst.tile([128, 1024], fp32, tag="st",
                                   name=f"psA_{j}_{sk}")
                    psB = pst.tile([128, 1024], fp32, tag="st",
                                   name=f"psB_{j}_{sk}")
                    for qc in range(2):
                        nc.tensor.matmul(
                            psA[:, qc * 512:(qc + 1) * 512],
                            kT[0:DK, j, sk * 128:(sk + 1) * 128],
                            qT[0:DK, j, qc * 512:(qc + 1) * 512],
                            start=True, stop=True, tile_position=(0, 0))
                    for qc in range(2):
                        nc.tensor.matmul(
                            psB[:, qc * 512:(qc + 1) * 512],
                            kT[DK:128, j, sk * 128:(sk + 1) * 128],
                            qT[DK:128, j, qc * 512:(qc + 1) * 512],
                            start=True, stop=True, tile_position=(64, 0))
                    stA = stp.tile([128, 1024], bf16, tag="stb",
                                   name=f"stA_{j}_{sk}")
                    nc.scalar.activation(out=stA[:], in_=psA[:], func=EXP,
                                         scale=0.125)
                    stB = stp.tile([128, 1024], bf16, tag="stb",
                                   name=f"stB_{j}_{sk}")
                    nc.scalar.activation(out=stB[:], in_=psB[:], func=EXP,
                                         scale=0.125)
                    sts.append((stA, stB))
                    # attnv for the PREVIOUS key tile (keeps tensor ahead
                    # of the scalar exp stream)
                    if sk > 0:
                        pA, pB = sts[sk - 1]
                        for qc in range(2):
                            nc.tensor.matmul(
                                po0[0:DK + 1, qc * 512:(qc + 1) * 512],
                                v_sb[:, sk - 1, 2 * j, :],
                                pA[:, qc * 512:(qc + 1) * 512],
                                start=(sk == 1), stop=False)
                        for qc in range(2):
                            nc.tensor.matmul(
                                po1[0:DK + 1, qc * 512:(qc + 1) * 512],
                                v_sb[:, sk - 1, 2 * j + 1, :],
                                pB[:, qc * 512:(qc + 1) * 512],
                                start=(sk == 1), stop=False)
                # flush last key tile
                pA, pB = sts[nkt - 1]
                for qc in range(2):
                    nc.tensor.matmul(
                        po0[0:DK + 1, qc * 512:(qc + 1) * 512],
                        v_sb[:, nkt - 1, 2 * j, :],
                        pA[:, qc * 512:(qc + 1) * 512],
                        start=False, stop=True)
                for qc in range(2):
                    nc.tensor.matmul(
                        po1[0:DK + 1, qc * 512:(qc + 1) * 512],
                        v_sb[:, nkt - 1, 2 * j + 1, :],
                        pB[:, qc * 512:(qc + 1) * 512],
                        start=False, stop=True)

                # ---- evacuate + normalize ----
                uA = small.tile([128, 1024], fp32, tag="u", name=f"uA_{j}")
                nc.vector.tensor_copy(out=uA[0:DK + 1, :],
                                      in_=po0[0:DK + 1, :])
                nc.sync.dma_start(out=rds[2 * j:2 * j + 1, :],
                                  in_=uA[DK:DK + 1, :])
                uB = small.tile([128, 1024], fp32, tag="u", name=f"uB_{j}")
                nc.vector.tensor_copy(out=uB[0:DK + 1, :],
                                      in_=po1[0:DK + 1, :])
                nc.sync.dma_start(out=rds[2 * j + 1:2 * j + 2, :],
                                  in_=uB[DK:DK + 1, :])
                den2 = small.tile([128, 16], fp32, tag="den",
                                  name=f"den_{j}")
                nc.gpsimd.dma_start(
                    out=den2[:],
                    in_=rds[2 * j:2 * j + 2, :]
                    .rearrange("h (t p) -> p h t", p=128))
                rec2 = small.tile([128, 16], fp32, tag="rec",
                                  name=f"rec_{j}")
                nc.vector.reciprocal_approx_fast(out=rec2[:], in_=den2[:])
                nc.sync.dma_start(
                    out=rds2[2 * j:2 * j + 2, :]
                    .rearrange("h (t p) -> p h t", p=128),
                    in_=rec2[:])
                bcA = small.tile([128, 1024], fp32, tag="bc",
                                 name=f"bcA_{j}")
                nc.gpsimd.dma_start(
                    out=bcA[0:DK, :],
                    in_=rds2[2 * j:2 * j + 1, :].partition_broadcast(DK))
                nc.vector.tensor_mul(out=outTn[0:DK, j, :],
                                     in0=uA[0:DK, :], in1=bcA[0:DK, :])
                bcB = small.tile([128, 1024], fp32, tag="bc",
                                 name=f"bcB_{j}")
                nc.gpsimd.dma_start(
                    out=bcB[0:DK, :],
                    in_=rds2[2 * j + 1:2 * j + 2, :].partition_broadcast(DK))
                todd = small.tile([DK, 1024], bf16, tag="todd",
                                  name=f"todd_{j}")
                nc.vector.tensor_mul(out=todd[:],
                                     in0=uB[0:DK, :], in1=bcB[0:DK, :])
                nc.sync.dma_start(out=outTn[DK:128, j, :], in_=todd[:])

            # ---- output projection: one wave, 8 single-bank accumulators --
            P = [pst.tile([128, 1024], fp32, tag="st", name="pfA"),
                 pst.tile([128, 1024], fp32, tag="st", name="pfB"),
                 pout.tile([128, 1024], fp32, tag="po", name="pfC"),
                 pout.tile([128, 1024], fp32, tag="po", name="pfD")]
            for jj in range(4):
                for k8 in range(8):
                    pf = P[k8 // 2][:, (k8 % 2) * 512:(k8 % 2 + 1) * 512]
                    nc.tensor.matmul(
                        pf,
                        outTn[:, jj, k8 * 128:(k8 + 1) * 128],
                        wo_sb[:, jj, :],
                        start=(jj == 0), stop=(jj == 3))
            rings = [nc.sync, nc.scalar, nc.gpsimd]
            for k8 in range(8):
                pf = P[k8 // 2][:, (k8 % 2) * 512:(k8 % 2 + 1) * 512]
                ob = osb.tile([128, 512], fp32, tag="ob", name=f"ob_{k8}")
                nc.vector.tensor_add(out=ob[:], in0=pf, in1=bo2_sb[:])
                rings[k8 % 3].dma_start(
                    out=out[k8 * 128:(k8 + 1) * 128, :], in_=ob[:])

    nc.finalize()
    return nc


def _get_nc(skeys):
    if skeys not in _compiled:
        _compiled[skeys] = _build(skeys)
    return _compiled[skeys]


def kernel(query, key, value, key_padding_mask, Wq, bq, Wk, bk, Wv, bv,
           Wo, bo):
    global last_results
    from concourse.bass_utils import run_bass_kernel_spmd
    import ml_dtypes
    bf = ml_dtypes.bfloat16

    query = np.asarray(query, dtype=np.float32)
    key = np.asarray(key, dtype=np.float32)
    value = np.asarray(value, dtype=np.float32)
    mask = np.asarray(key_padding_mask).astype(bool)
    Wq = np.asarray(Wq, dtype=np.float32)
    Wk = np.asarray(Wk, dtype=np.float32)
    Wv = np.asarray(Wv, dtype=np.float32)
    Wo = np.asarray(Wo, dtype=np.float32)
    bqv = np.asarray(bq, dtype=np.float32)
    bvv = np.asarray(bv, dtype=np.float32)
    bov = np.asarray(bo, dtype=np.float32)

    # compact keys: keep only unmasked positions (zero-padded to SKC);
    # dense fallback when a batch keeps more than SKC. Masked/padded keys
    # carry zero v and a zero "ones" entry -> no mask bias needed anywhere.
    kept = [np.flatnonzero(~mask[b]) for b in range(B)]
    if max(len(k) for k in kept) <= SKC:
        skeys = SKC
        kc_l, vc_l, ko_l = [], [], []
        for b in range(B):
            n = len(kept[b])
            kc = np.zeros((skeys, D), np.float32)
            vc = np.zeros((skeys, D), np.float32)
            kc[:n] = key[b][kept[b]]
            vc[:n] = value[b][kept[b]]
            kones = np.zeros(skeys, np.float32)
            kones[:n] = 1.0
            kc_l.append(kc); vc_l.append(vc); ko_l.append(kones)
    else:
        skeys = S
        kc_l = [key[b] for b in range(B)]
        vc_l = [value[b] * (~mask[b])[:, None] for b in range(B)]
        ko_l = [(~mask[b]).astype(np.float32) for b in range(B)]

    nc = _get_nc(skeys)
    nkt = skeys // 128

    bo2 = bov + Wo @ bvv  # fold v bias through the output projection
    shared = {
        "wq": np.ascontiguousarray(Wq.T).astype(bf),
        "wk": np.ascontiguousarray(Wk.T).astype(bf),
        "wv": np.ascontiguousarray(Wv.T).astype(bf),
        "wo": np.ascontiguousarray(Wo.T).astype(bf),
        "bq": np.ascontiguousarray(bqv.reshape(4, 128).T),
        "bo2": bo2.reshape(1, D).astype(np.float32),
    }
    in_maps = []
    for c in range(N_CORES):
        b, qh = divmod(c, 2)
        qT = np.ascontiguousarray(query[b].T)
        m = {
            "xq": np.ascontiguousarray(
                qT[:, qh * SQ:(qh + 1) * SQ]).astype(bf),
            "xk": np.ascontiguousarray(kc_l[b].T).astype(bf),
            "xv": np.ascontiguousarray(vc_l[b].T).astype(bf),
            "ko": np.ascontiguousarray(
                ko_l[b].reshape(nkt, 128).T).astype(bf),
        }
        m.update(shared)
        in_maps.append(m)

    res = run_bass_kernel_spmd(nc, in_maps, list(range(N_CORES)))
    last_results = res

    out = np.empty((B, S, D), dtype=np.float32)
    for c in range(N_CORES):
        b, qh = divmod(c, 2)
        out[b, qh * SQ:(qh + 1) * SQ, :] = res.results[c]["out"]
    return out


# revision 7
# speedup vs baseline: 1.3049x; 1.3049x over previous
# Multi-head attention (B=4, S=2048, D=512, H=8) on 8 Trainium2 NeuronCores.
#
# Sharding: core c handles batch c//2 and query rows [(c%2)*1024, (c%2+1)*1024)
# for all 8 heads over all 2048 keys. Output slices are disjoint -> no
# collectives needed.
#
# v2 design notes (vs the v1 baseline at 188us):
#   - mask machinery removed entirely: masked/padded keys ship ZERO xk/xv
#     columns and a 0 entry in the "kones" vector that fills the denominator
#     row of the v stationary. exp(q.0)=1 but both its v row and its ones
#     entry are 0, so padded keys contribute nothing to numerator or
#     denominator. Works for the compacted AND the dense fallback layout.
#   - k bias dropped (constant-per-query term, cancels exactly in softmax).
#     v bias + o bias folded on host: bo2 = bo + Wo @ bv (softmax rows sum
#     to 1, so the bv term rides through attention unchanged).
#   - v projection interleaved into the j=0 attention loop so the scalar
#     engine's exp stream (the true bottleneck, ~72 x 1us) starts ~15us
#     earlier and vproj hides under it.
#   - odd heads store v as [1 | v] so their attnv PSUM lands at partitions
#     63..127: the normalize multiply writes outTn[64:128] lane-aligned,
#     killing v1's 4 SBUF->SBUF shuffle DMAs.
#   - reciprocal: v1 spent 6.5us per [1,1024] DVE reciprocal (52us total!).
#     Now the two denominators of a head pair round-trip through DRAM into a
#     [128,16] tile and use one reciprocal_approx_fast (~0.2us).
#   - output projection runs in ONE wave: 8 x [128,512] single-bank PSUM
#     accumulators, jj-outer, so jj=0..2 passes overlap the last pair's
#     normalization; bias is added by the DVE during PSUM evacuation.
#   - PE HAM warmup: ~10 dummy matmuls during the initial DMA wait so the
#     clock gate is at 8/8 when real matmuls arrive; exp table preloaded the
#     same way.
#   - input DMAs spread across scalar/sync/vector/gpsimd rings, output DMAs
#     round-robin over 4 rings.

import sys
import os

for _p in ("/opt/trn_rl_repo", "/root/.axon_site/_ro/trn_rl_repo"):
    if os.path.isdir(_p) and _p not in sys.path:
        sys.path.append(_p)

import numpy as np

B, S, D, H = 4, 2048, 512, 8
DK = D // H          # 64
N_CORES = 8
SQ = S // 2          # 1024 query rows per core
SKC = 1152           # compacted key capacity (9 tiles of 128)

_compiled = {}       # skeys -> Bacc
last_results = None  # BassKernelResults of the most recent run (for test.py)


def _build(skeys):
    import concourse.bass as bass  # noqa: F401
    from concourse import bacc
    import concourse.tile as tile
    import concourse.mybir as mybir

    fp32 = mybir.dt.float32
    bf16 = mybir.dt.bfloat16
    EXP = mybir.ActivationFunctionType.Exp
    nkt = skeys // 128
    # key-side projection chunks of up to 512 columns
    kchunks = []
    off = 0
    while off < skeys:
        w = min(512, skeys - off)
        kchunks.append((off, w))
        off += w

    nc = bacc.Bacc("TRN2", target_bir_lowering=False, debug=False,
                   num_devices=N_CORES)

    xq = nc.dram_tensor("xq", [D, SQ], bf16, kind="ExternalInput")
    xk = nc.dram_tensor("xk", [D, skeys], bf16, kind="ExternalInput")
    xv = nc.dram_tensor("xv", [D, skeys], bf16, kind="ExternalInput")
    wq = nc.dram_tensor("wq", [D, D], bf16, kind="ExternalInput")
    wk = nc.dram_tensor("wk", [D, D], bf16, kind="ExternalInput")
    wv = nc.dram_tensor("wv", [D, D], bf16, kind="ExternalInput")
    wo = nc.dram_tensor("wo", [D, D], bf16, kind="ExternalInput")
    bq = nc.dram_tensor("bq", [128, 4], fp32, kind="ExternalInput")
    ko = nc.dram_tensor("ko", [128, nkt], bf16, kind="ExternalInput")
    bo2 = nc.dram_tensor("bo2", [1, D], fp32, kind="ExternalInput")
    out = nc.dram_tensor("out", [SQ, D], fp32, kind="ExternalOutput")
    rds = nc.dram_tensor("rds", [H, SQ], fp32)   # scratch: denominators
    rds2 = nc.dram_tensor("rds2", [H, SQ], fp32)  # scratch: 1/denominator

    with tile.TileContext(nc) as tc:
        with (
            tc.tile_pool(name="consts", bufs=1) as consts,
            tc.tile_pool(name="xin", bufs=1) as xin,
            tc.tile_pool(name="qk", bufs=1) as qk,
            tc.tile_pool(name="vp", bufs=1) as vp,
            tc.tile_pool(name="stp", bufs=4) as stp,
            tc.tile_pool(name="small", bufs=2) as small,
            tc.tile_pool(name="osb", bufs=8) as osb,
            tc.tile_pool(name="pst", bufs=2, space="PSUM") as pst,
            tc.tile_pool(name="pout", bufs=2, space="PSUM") as pout,
        ):
            # ---- warmup: exp table load + PE HAM un-throttle ----
            warm = consts.tile([128, 512], bf16, tag="warm")
            nc.vector.memset(warm[:], 0.25)
            warm_act = consts.tile([1, 512], bf16, tag="warma")
            nc.scalar.activation(out=warm_act[:], in_=warm[0:1, :], func=EXP,
                                 scale=1.0)
            wps = pst.tile([128, 1024], fp32, tag="st", name="warm_ps")
            for _ in range(10):
                nc.tensor.matmul(wps[:, 0:512], warm[:, 0:128],
                                 warm[:, 0:512], start=True, stop=True)

            # ---- input DMAs, spread across rings ----
            # scalar: wq, wk, wo | sync: xq | vector: xk, xv | gpsimd: rest
            wq_sb = consts.tile([128, 4, D], bf16, tag="wq")
            for kc in range(4):
                nc.scalar.dma_start(out=wq_sb[:, kc, :],
                                    in_=wq[kc * 128:(kc + 1) * 128, :])
            xq_sb = xin.tile([128, 4, SQ], bf16, tag="xq")
            nc.sync.dma_start(
                out=xq_sb[:],
                in_=xq.rearrange("(kc p) s -> p kc s", p=128))
            xk_sb = xin.tile([128, 4, skeys], bf16, tag="xk")
            for off, w in kchunks:
                nc.gpsimd.dma_start(
                    out=xk_sb[:, :, off:off + w],
                    in_=xk[:, off:off + w]
                    .rearrange("(kc p) s -> p kc s", p=128))
            wk_sb = consts.tile([128, 4, D], bf16, tag="wk")
            for kc in range(4):
                nc.scalar.dma_start(out=wk_sb[:, kc, :],
                                    in_=wk[kc * 128:(kc + 1) * 128, :])
            bq_sb = consts.tile([128, 4], fp32, tag="bq")
            nc.gpsimd.dma_start(out=bq_sb[:], in_=bq[:, :])
            ko_sb = consts.tile([128, nkt], bf16, tag="ko")
            nc.gpsimd.dma_start(out=ko_sb[:], in_=ko[:, :])
            wv_sb = consts.tile([128, 4, D], bf16, tag="wv")
            for kc in range(4):
                nc.scalar.dma_start(out=wv_sb[:, kc, :],
                                    in_=wv[kc * 128:(kc + 1) * 128, :])
            xv_sb = xin.tile([128, 4, skeys], bf16, tag="xv")
            for off, w in kchunks:
                nc.sync.dma_start(
                    out=xv_sb[:, :, off:off + w],
                    in_=xv[:, off:off + w]
                    .rearrange("(kc p) s -> p kc s", p=128))
            # WoT rows packed by head pair: [128, 4, 512]
            wo_sb = consts.tile([128, 4, D], bf16, tag="wo")
            nc.scalar.dma_start(out=wo_sb[:],
                                in_=wo.rearrange("(j p) n -> p j n", p=128))
            bo2_sb = consts.tile([128, D], fp32, tag="bo2")
            nc.gpsimd.dma_start(out=bo2_sb[:],
                                in_=bo2[0:1, :].partition_broadcast(128))

            # ---- q projection (head pairs stacked on partitions) ----
            # stationary per (j,kc) streams both query halves -> 16 LDWs
            qT = qk.tile([128, 4, SQ], bf16, tag="qT")
            for j in range(4):
                pqA = pst.tile([128, 1024], fp32, tag="st", name=f"pqA_{j}")
                pqB = pst.tile([128, 1024], fp32, tag="st", name=f"pqB_{j}")
                for kc in range(4):
                    for qc, p in ((0, pqA), (1, pqB)):
                        nc.tensor.matmul(
                            p[:, 0:512],
                            wq_sb[:, kc, j * 128:(j + 1) * 128],
                            xq_sb[:, kc, qc * 512:(qc + 1) * 512],
                            start=(kc == 0), stop=(kc == 3))
                nc.scalar.add(qT[:, j, 0:512], pqA[:, 0:512],
                              bq_sb[:, j:j + 1])
                nc.scalar.add(qT[:, j, 512:1024], pqB[:, 0:512],
                              bq_sb[:, j:j + 1])

            # ---- k projection (no bias: cancels in softmax) ----
            kT = qk.tile([128, 4, skeys], bf16, tag="kT")
            # group the <=512 chunks in pairs per [128,1024] psum tile
            cgroups = [kchunks[i:i + 2] for i in range(0, len(kchunks), 2)]
            for j in range(4):
                for gi, grp in enumerate(cgroups):
                    pk = pst.tile([128, 1024], fp32, tag="st",
                                  name=f"pk_{j}_{gi}")
                    for kc in range(4):
                        for ci, (off, w) in enumerate(grp):
                            nc.tensor.matmul(
                                pk[:, ci * 512:ci * 512 + w],
                                wk_sb[:, kc, j * 128:(j + 1) * 128],
                                xk_sb[:, kc, off:off + w],
                                start=(kc == 0), stop=(kc == 3))
                    for ci, (off, w) in enumerate(grp):
                        nc.scalar.copy(kT[:, j, off:off + w],
                                       pk[:, ci * 512:ci * 512 + w])

            # ---- v stationaries: [v | kones] per head ----
            v_sb = vp.tile([128, nkt, H, DK + 1], bf16, tag="v")
            for h in range(H):
                nc.vector.tensor_copy(out=v_sb[:, :, h, DK], in_=ko_sb[:, :])
            for sk in range(nkt):
                pv = pst.tile([128, 1024], fp32, tag="st", name=f"pv_{sk}")
                for kc in range(4):
                    nc.tensor.matmul(
                        pv[:, 0:512],
                        xv_sb[:, kc, sk * 128:(sk + 1) * 128],
                        wv_sb[:, kc, :],
                        start=(kc == 0), stop=(kc == 3))
                nc.vector.tensor_copy(
                    out=v_sb[:, sk, :, 0:DK],
                    in_=pv[:, 0:512].rearrange("p (h m) -> p h m", h=H))

            # ---- attention, one head pair at a time ----
            # per sk: scores (A then B), exp on scalar, attnv of sk-1.
            # v projection for key tile sk interleaved into the j=0 loop.
            outTn = qk.tile([128, 4, SQ], bf16, tag="outTn")
            for j in range(4):
                po0 = pout.tile([128, 1024], fp32, tag="po", name=f"po0_{j}")
                po1 = pout.tile([128, 1024], fp32, tag="po", name=f"po1_{j}")
                # software pipeline: in the tensor FIFO, scores(sk)
                # precede attnv(sk-1) per half, so the next exp's input is
                # always ready the moment its WAR slot frees and the scalar
                # engine never starves.
                sts = []

                def attnv(sk, half):
                    pX = sts[sk][half]
                    po = po0 if half == 0 else po1
                    for qc in range(2):
                        nc.tensor.matmul(
                            po[0:DK + 1, qc * 512:(qc + 1) * 512],
                            v_sb[:, sk, 2 * j + half, :],
                            pX[:, qc * 512:(qc + 1) * 512],
                            start=(sk == 0), stop=(sk == nkt - 1))

                for sk in range(nkt):
                    psA = pst.tile([128, 1024], fp32, tag="st",
                                   name=f"psA_{j}_{sk}")
                    psB = pst.tile([128, 1024], fp32, tag="st",
                                   name=f"psB_{j}_{sk}")
                    for qc in range(2):
                        nc.tensor.matmul(
                            psA[:, qc * 512:(qc + 1) * 512],
                            kT[0:DK, j, sk * 128:(sk + 1) * 128],
                            qT[0:DK, j, qc * 512:(qc + 1) * 512],
                            start=True, stop=True, tile_position=(0, 0))
                    if sk > 0:
                        attnv(sk - 1, 0)
                    for qc in range(2):
                        nc.tensor.matmul(
                            psB[:, qc * 512:(qc + 1) * 512],
                            kT[DK:128, j, sk * 128:(sk + 1) * 128],
                            qT[DK:128, j, qc * 512:(qc + 1) * 512],
                            start=True, stop=True, tile_position=(64, 0))
                    if sk > 0:
                        attnv(sk - 1, 1)
                    stA = stp.tile([128, 1024], bf16, tag="stb",
                                   name=f"stA_{j}_{sk}")
                    nc.scalar.activation(out=stA[:], in_=psA[:], func=EXP,
                                         scale=0.125)
                    stB = stp.tile([128, 1024], bf16, tag="stb",
                                   name=f"stB_{j}_{sk}")
                    nc.scalar.activation(out=stB[:], in_=psB[:], func=EXP,
                                         scale=0.125)
                    sts.append((stA, stB))
                attnv(nkt - 1, 0)
                attnv(nkt - 1, 1)

                # ---- evacuate + normalize ----
                # 1/den straight off the PSUM den row (reciprocal_approx_fast
                # is single-pass, ~1.2us on one lane), DRAM round trip only
                # for the partition broadcast; all DMAs contiguous.
                uA = small.tile([128, 1024], fp32, tag="u", name=f"uA_{j}")
                nc.vector.tensor_copy(out=uA[0:DK + 1, :],
                                      in_=po0[0:DK + 1, :])
                nc.sync.dma_start(out=rds[2 * j:2 * j + 1, :],
                                  in_=uA[DK:DK + 1, :])
                uB = small.tile([128, 1024], fp32, tag="u", name=f"uB_{j}")
                nc.vector.tensor_copy(out=uB[0:DK + 1, :],
                                      in_=po1[0:DK + 1, :])
                nc.sync.dma_start(out=rds[2 * j + 1:2 * j + 2, :],
                                  in_=uB[DK:DK + 1, :])
                # gather both den rows as [32, 2, 32] (128B runs), recip,
                # scatter back with the same block mapping (rds2 stays in
                # natural q order for the partition_broadcast below)
                den2 = small.tile([32, 2, 32], fp32, tag="den",
                                  name=f"den_{j}")
                nc.gpsimd.dma_start(
                    out=den2[:],
                    in_=rds[2 * j:2 * j + 2, :]
                    .rearrange("h (p t) -> p h t", p=32))
                rec2 = small.tile([32, 2, 32], fp32, tag="rec",
                                  name=f"rec_{j}")
                nc.vector.reciprocal_approx_fast(out=rec2[:], in_=den2[:])
                nc.sync.dma_start(
                    out=rds2[2 * j:2 * j + 2, :]
                    .rearrange("h (p t) -> p h t", p=32),
                    in_=rec2[:])
                bcA = small.tile([128, 1024], fp32, tag="bc",
                                 name=f"bcA_{j}")
                nc.gpsimd.dma_start(
                    out=bcA[0:DK, :],
                    in_=rds2[2 * j:2 * j + 1, :].partition_broadcast(DK))
                nc.vector.tensor_mul(out=outTn[0:DK, j, :],
                                     in0=uA[0:DK, :], in1=bcA[0:DK, :])
                bcB = small.tile([128, 1024], fp32, tag="bc",
                                 name=f"bcB_{j}")
                nc.gpsimd.dma_start(
                    out=bcB[0:DK, :],
                    in_=rds2[2 * j + 1:2 * j + 2, :].partition_broadcast(DK))
                todd = small.tile([DK, 1024], bf16, tag="todd",
                                  name=f"todd_{j}")
                nc.vector.tensor_mul(out=todd[:],
                                     in0=uB[0:DK, :], in1=bcB[0:DK, :])
                nc.sync.dma_start(out=outTn[DK:128, j, :], in_=todd[:])

            # ---- output projection: one wave, 8 single-bank accumulators --
            P = [pst.tile([128, 1024], fp32, tag="st", name="pfA"),
                 pst.tile([128, 1024], fp32, tag="st", name="pfB"),
                 pout.tile([128, 1024], fp32, tag="po", name="pfC"),
                 pout.tile([128, 1024], fp32, tag="po", name="pfD")]
            for jj in range(4):
                for k8 in range(8):
                    pf = P[k8 // 2][:, (k8 % 2) * 512:(k8 % 2 + 1) * 512]
                    nc.tensor.matmul(
                        pf,
                        outTn[:, jj, k8 * 128:(k8 + 1) * 128],
                        wo_sb[:, jj, :],
                        start=(jj == 0), stop=(jj == 3))
            rings = [nc.sync, nc.scalar, nc.gpsimd]
            for k8 in range(8):
                pf = P[k8 // 2][:, (k8 % 2) * 512:(k8 % 2 + 1) * 512]
                ob = osb.tile([128, 512], fp32, tag="ob", name=f"ob_{k8}")
                nc.vector.tensor_add(out=ob[:], in0=pf, in1=bo2_sb[:])
                rings[k8 % 3].dma_start(
                    out=out[k8 * 128:(k8 + 1) * 128, :], in_=ob[:])

    nc.finalize()
    return nc


def _get_nc(skeys):
    if skeys not in _compiled:
        _compiled[skeys] = _build(skeys)
    return _compiled[skeys]


def kernel(query, key, value, key_padding_mask, Wq, bq, Wk, bk, Wv, bv,
           Wo, bo):
    global last_results
    from concourse.bass_utils import run_bass_kernel_spmd
    import ml_dtypes
    bf = ml_dtypes.bfloat16

    query = np.asarray(query, dtype=np.float32)
    key = np.asarray(key, dtype=np.float32)
    value = np.asarray(value, dtype=np.float32)
    mask = np.asarray(key_padding_mask).astype(bool)
    Wq = np.asarray(Wq, dtype=np.float32)
    Wk = np.asarray(Wk, dtype=np.float32)
    Wv = np.asarray(Wv, dtype=np.float32)
    Wo = np.asarray(Wo, dtype=np.float32)
    bqv = np.asarray(bq, dtype=np.float32)
    bvv = np.asarray(bv, dtype=np.float32)
    bov = np.asarray(bo, dtype=np.float32)

    # compact keys: keep only unmasked positions (zero-padded to SKC);
    # dense fallback when a batch keeps more than SKC. Masked/padded keys
    # carry zero v and a zero "ones" entry -> no mask bias needed anywhere.
    kept = [np.flatnonzero(~mask[b]) for b in range(B)]
    if max(len(k) for k in kept) <= SKC:
        skeys = SKC
        kc_l, vc_l, ko_l = [], [], []
        for b in range(B):
            n = len(kept[b])
            kc = np.zeros((skeys, D), np.float32)
            vc = np.zeros((skeys, D), np.float32)
            kc[:n] = key[b][kept[b]]
            vc[:n] = value[b][kept[b]]
            kones = np.zeros(skeys, np.float32)
            kones[:n] = 1.0
            kc_l.append(kc); vc_l.append(vc); ko_l.append(kones)
    else:
        skeys = S
        kc_l = [key[b] for b in range(B)]
        vc_l = [value[b] * (~mask[b])[:, None] for b in range(B)]
        ko_l = [(~mask[b]).astype(np.float32) for b in range(B)]

    nc = _get_nc(skeys)
    nkt = skeys // 128

    bo2 = bov + Wo @ bvv  # fold v bias through the output projection
    shared = {
        "wq": np.ascontiguousarray(Wq.T).astype(bf),
        "wk": np.ascontiguousarray(Wk.T).astype(bf),
        "wv": np.ascontiguousarray(Wv.T).astype(bf),
        "wo": np.ascontiguousarray(Wo.T).astype(bf),
        "bq": np.ascontiguousarray(bqv.reshape(4, 128).T),
        "bo2": bo2.reshape(1, D).astype(np.float32),
    }
    in_maps = []
    for c in range(N_CORES):
        b, qh = divmod(c, 2)
        qT = np.ascontiguousarray(query[b].T)
        m = {
            "xq": np.ascontiguousarray(
                qT[:, qh * SQ:(qh + 1) * SQ]).astype(bf),
            "xk": np.ascontiguousarray(kc_l[b].T).astype(bf),
            "xv": np.ascontiguousarray(vc_l[b].T).astype(bf),
            "ko": np.ascontiguousarray(
                ko_l[b].reshape(nkt, 128).T).astype(bf),
        }
        m.update(shared)
        in_maps.append(m)

    res = run_bass_kernel_spmd(nc, in_maps, list(range(N_CORES)))
    last_results = res

    out = np.empty((B, S, D), dtype=np.float32)
    for c in range(N_CORES):
        b, qh = divmod(c, 2)
        out[b, qh * SQ:(qh + 1) * SQ, :] = res.results[c]["out"]
    return out


# revision 10
# speedup vs baseline: 1.3903x; 1.0655x over previous
# Multi-head attention (B=4, S=2048, D=512, H=8) on 8 Trainium2 NeuronCores.
#
# Sharding: core c handles batch c//2 and query rows [(c%2)*1024, (c%2+1)*1024)
# for all 8 heads over all 2048 keys. Output slices are disjoint -> no
# collectives needed.
#
# v2 design notes (vs the v1 baseline at 188us):
#   - mask machinery removed entirely: masked/padded keys ship ZERO xk/xv
#     columns and a 0 entry in the "kones" vector that fills the denominator
#     row of the v stationary. exp(q.0)=1 but both its v row and its ones
#     entry are 0, so padded keys contribute nothing to numerator or
#     denominator. Works for the compacted AND the dense fallback layout.
#   - k bias dropped (constant-per-query term, cancels exactly in softmax).
#     v bias + o bias folded on host: bo2 = bo + Wo @ bv (softmax rows sum
#     to 1, so the bv term rides through attention unchanged).
#   - v projection interleaved into the j=0 attention loop so the scalar
#     engine's exp stream (the true bottleneck, ~72 x 1us) starts ~15us
#     earlier and vproj hides under it.
#   - odd heads store v as [1 | v] so their attnv PSUM lands at partitions
#     63..127: the normalize multiply writes outTn[64:128] lane-aligned,
#     killing v1's 4 SBUF->SBUF shuffle DMAs.
#   - reciprocal: v1 spent 6.5us per [1,1024] DVE reciprocal (52us total!).
#     Now the two denominators of a head pair round-trip through DRAM into a
#     [128,16] tile and use one reciprocal_approx_fast (~0.2us).
#   - output projection runs in ONE wave: 8 x [128,512] single-bank PSUM
#     accumulators, jj-outer, so jj=0..2 passes overlap the last pair's
#     normalization; bias is added by the DVE during PSUM evacuation.
#   - PE HAM warmup: ~10 dummy matmuls during the initial DMA wait so the
#     clock gate is at 8/8 when real matmuls arrive; exp table preloaded the
#     same way.
#   - input DMAs spread across scalar/sync/vector/gpsimd rings, output DMAs
#     round-robin over 4 rings.

import sys
import os

for _p in ("/opt/trn_rl_repo", "/root/.axon_site/_ro/trn_rl_repo"):
    if os.path.isdir(_p) and _p not in sys.path:
        sys.path.append(_p)

import numpy as np

B, S, D, H = 4, 2048, 512, 8
DK = D // H          # 64
N_CORES = 8
SQ = S // 2          # 1024 query rows per core
SKC = 1152           # compacted key capacity (9 tiles of 128)

_compiled = {}       # skeys -> Bacc
last_results = None  # BassKernelResults of the most recent run (for test.py)


def _build(skeys):
    import concourse.bass as bass  # noqa: F401
    from concourse import bacc
    import concourse.tile as tile
    import concourse.mybir as mybir

    fp32 = mybir.dt.float32
    bf16 = mybir.dt.bfloat16
    EXP = mybir.ActivationFunctionType.Exp
    nkt = skeys // 128
    # key-side projection chunks of up to 512 columns
    kchunks = []
    off = 0
    while off < skeys:
        w = min(512, skeys - off)
        kchunks.append((off, w))
        off += w

    nc = bacc.Bacc("TRN2", target_bir_lowering=False, debug=False,
                   num_devices=N_CORES)

    xq = nc.dram_tensor("xq", [D, SQ], bf16, kind="ExternalInput")
    xk = nc.dram_tensor("xk", [D, skeys], bf16, kind="ExternalInput")
    xv = nc.dram_tensor("xv", [D, skeys], bf16, kind="ExternalInput")
    wq = nc.dram_tensor("wq", [D, D], bf16, kind="ExternalInput")
    wk = nc.dram_tensor("wk", [D, D], bf16, kind="ExternalInput")
    wv = nc.dram_tensor("wv", [D, D], bf16, kind="ExternalInput")
    wo = nc.dram_tensor("wo", [D, D], bf16, kind="ExternalInput")
    bq = nc.dram_tensor("bq", [128, 4], fp32, kind="ExternalInput")
    ko = nc.dram_tensor("ko", [128, nkt], bf16, kind="ExternalInput")
    bo2 = nc.dram_tensor("bo2", [1, D], fp32, kind="ExternalInput")
    out = nc.dram_tensor("out", [SQ, D], fp32, kind="ExternalOutput")
    rds = nc.dram_tensor("rds", [H, SQ], fp32)   # scratch: denominators
    rds2 = nc.dram_tensor("rds2", [H, SQ], fp32)  # scratch: 1/denominator

    with tile.TileContext(nc) as tc:
        with (
            tc.tile_pool(name="consts", bufs=1) as consts,
            tc.tile_pool(name="xin", bufs=1) as xin,
            tc.tile_pool(name="qk", bufs=1) as qk,
            tc.tile_pool(name="vp", bufs=1) as vp,
            tc.tile_pool(name="stp", bufs=4) as stp,
            tc.tile_pool(name="small", bufs=2) as small,
            tc.tile_pool(name="osb", bufs=8) as osb,
            tc.tile_pool(name="pst", bufs=2, space="PSUM") as pst,
            tc.tile_pool(name="pout", bufs=2, space="PSUM") as pout,
        ):
            # ---- warmup: exp table load + PE HAM un-throttle ----
            warm = consts.tile([128, 512], bf16, tag="warm")
            nc.vector.memset(warm[:], 0.25)
            warm_act = consts.tile([1, 512], bf16, tag="warma")
            nc.scalar.activation(out=warm_act[:], in_=warm[0:1, :], func=EXP,
                                 scale=1.0)
            wps = pst.tile([128, 1024], fp32, tag="st", name="warm_ps")
            for _ in range(10):
                nc.tensor.matmul(wps[:, 0:512], warm[:, 0:128],
                                 warm[:, 0:512], start=True, stop=True)

            # ---- input DMAs, spread across rings ----
            # scalar: wq, wk, wo | sync: xq | vector: xk, xv | gpsimd: rest
            wq_sb = consts.tile([128, 4, D], bf16, tag="wq")
            nc.scalar.dma_start(out=wq_sb[:],
                                in_=wq.rearrange("(kc p) n -> p kc n", p=128))
            xq_sb = xin.tile([128, 4, SQ], bf16, tag="xq")
            nc.sync.dma_start(
                out=xq_sb[:],
                in_=xq.rearrange("(kc p) s -> p kc s", p=128))
            xk_sb = xin.tile([128, 4, skeys], bf16, tag="xk")
            for off, w in kchunks:
                nc.gpsimd.dma_start(
                    out=xk_sb[:, :, off:off + w],
                    in_=xk[:, off:off + w]
                    .rearrange("(kc p) s -> p kc s", p=128))
            wk_sb = consts.tile([128, 4, D], bf16, tag="wk")
            nc.scalar.dma_start(out=wk_sb[:],
                                in_=wk.rearrange("(kc p) n -> p kc n", p=128))
            bq_sb = consts.tile([128, 4], fp32, tag="bq")
            nc.gpsimd.dma_start(out=bq_sb[:], in_=bq[:, :])
            ko_sb = consts.tile([128, nkt], bf16, tag="ko")
            nc.gpsimd.dma_start(out=ko_sb[:], in_=ko[:, :])
            wv_sb = consts.tile([128, 4, D], bf16, tag="wv")
            nc.scalar.dma_start(out=wv_sb[:],
                                in_=wv.rearrange("(kc p) n -> p kc n", p=128))
            xv_sb = xin.tile([128, 4, skeys], bf16, tag="xv")
            for off, w in kchunks:
                nc.sync.dma_start(
                    out=xv_sb[:, :, off:off + w],
                    in_=xv[:, off:off + w]
                    .rearrange("(kc p) s -> p kc s", p=128))
            # WoT rows packed by head pair: [128, 4, 512]
            wo_sb = consts.tile([128, 4, D], bf16, tag="wo")
            nc.scalar.dma_start(out=wo_sb[:],
                                in_=wo.rearrange("(j p) n -> p j n", p=128))
            bo2_sb = consts.tile([128, D], fp32, tag="bo2")
            nc.gpsimd.dma_start(out=bo2_sb[:],
                                in_=bo2[0:1, :].partition_broadcast(128))

            # ---- q/k projections, interleaved per head pair ----
            # qproj psum on the "st" ring (scalar evacuates with the bias
            # add); kproj psum on the "po" ring (DVE evacuates) -> the two
            # rings double-buffer each other and the tensor engine never
            # waits for an evacuation.
            qT = qk.tile([128, 4, SQ], bf16, tag="qT")
            kT = qk.tile([128, 4, skeys], bf16, tag="kT")
            cgroups = [kchunks[i:i + 2] for i in range(0, len(kchunks), 2)]
            for j in range(4):
                pqA = pst.tile([128, 1024], fp32, tag="st", name=f"pqA_{j}")
                pqB = pst.tile([128, 1024], fp32, tag="st", name=f"pqB_{j}")
                for kc in range(4):
                    for qc, p in ((0, pqA), (1, pqB)):
                        nc.tensor.matmul(
                            p[:, 0:512],
                            wq_sb[:, kc, j * 128:(j + 1) * 128],
                            xq_sb[:, kc, qc * 512:(qc + 1) * 512],
                            start=(kc == 0), stop=(kc == 3))
                nc.scalar.add(qT[:, j, 0:512], pqA[:, 0:512],
                              bq_sb[:, j:j + 1])
                nc.scalar.add(qT[:, j, 512:1024], pqB[:, 0:512],
                              bq_sb[:, j:j + 1])
                for gi, grp in enumerate(cgroups):
                    pk = pout.tile([128, 1024], fp32, tag="po",
                                   name=f"pk_{j}_{gi}")
                    for kc in range(4):
                        for ci, (off, w) in enumerate(grp):
                            nc.tensor.matmul(
                                pk[:, ci * 512:ci * 512 + w],
                                wk_sb[:, kc, j * 128:(j + 1) * 128],
                                xk_sb[:, kc, off:off + w],
                                start=(kc == 0), stop=(kc == 3))
                    for ci, (off, w) in enumerate(grp):
                        nc.vector.tensor_copy(out=kT[:, j, off:off + w],
                                              in_=pk[:, ci * 512:ci * 512 + w])

            # ---- v stationaries: [v | kones] per head ----
            v_sb = vp.tile([128, nkt, H, DK + 1], bf16, tag="v")
            for h in range(H):
                nc.vector.tensor_copy(out=v_sb[:, :, h, DK], in_=ko_sb[:, :])
            for sk in range(nkt):
                pv = pst.tile([128, 1024], fp32, tag="st", name=f"pv_{sk}")
                for kc in range(4):
                    nc.tensor.matmul(
                        pv[:, 0:512],
                        xv_sb[:, kc, sk * 128:(sk + 1) * 128],
                        wv_sb[:, kc, :],
                        start=(kc == 0), stop=(kc == 3))
                nc.vector.tensor_copy(
                    out=v_sb[:, sk, :, 0:DK],
                    in_=pv[:, 0:512].rearrange("p (h m) -> p h m", h=H))

            # ---- attention, one head pair at a time ----
            # per sk: scores (A then B), exp on scalar, attnv of sk-1.
            # v projection for key tile sk interleaved into the j=0 loop.
            outTn = qk.tile([128, 4, SQ], bf16, tag="outTn")
            for j in range(4):
                po0 = pout.tile([128, 1024], fp32, tag="po", name=f"po0_{j}")
                po1 = pout.tile([128, 1024], fp32, tag="po", name=f"po1_{j}")
                # software pipeline: in the tensor FIFO, scores(sk)
                # precede attnv(sk-1) per half, so the next exp's input is
                # always ready the moment its WAR slot frees and the scalar
                # engine never starves.
                sts = []

                def attnv(sk, half):
                    pX = sts[sk][half]
                    po = po0 if half == 0 else po1
                    for qc in range(2):
                        nc.tensor.matmul(
                            po[0:DK + 1, qc * 512:(qc + 1) * 512],
                            v_sb[:, sk, 2 * j + half, :],
                            pX[:, qc * 512:(qc + 1) * 512],
                            start=(sk == 0), stop=(sk == nkt - 1))

                for sk in range(nkt):
                    psA = pst.tile([128, 1024], fp32, tag="st",
                                   name=f"psA_{j}_{sk}")
                    psB = pst.tile([128, 1024], fp32, tag="st",
                                   name=f"psB_{j}_{sk}")
                    for qc in range(2):
                        nc.tensor.matmul(
                            psA[:, qc * 512:(qc + 1) * 512],
                            kT[0:DK, j, sk * 128:(sk + 1) * 128],
                            qT[0:DK, j, qc * 512:(qc + 1) * 512],
                            start=True, stop=True, tile_position=(0, 0))
                    if sk > 0:
                        attnv(sk - 1, 0)
                    for qc in range(2):
                        nc.tensor.matmul(
                            psB[:, qc * 512:(qc + 1) * 512],
                            kT[DK:128, j, sk * 128:(sk + 1) * 128],
                            qT[DK:128, j, qc * 512:(qc + 1) * 512],
                            start=True, stop=True, tile_position=(64, 0))
                    if sk > 0:
                        attnv(sk - 1, 1)
                    stA = stp.tile([128, 1024], bf16, tag="stb",
                                   name=f"stA_{j}_{sk}")
                    nc.scalar.activation(out=stA[:], in_=psA[:], func=EXP,
                                         scale=0.125)
                    stB = stp.tile([128, 1024], bf16, tag="stb",
                                   name=f"stB_{j}_{sk}")
                    nc.scalar.activation(out=stB[:], in_=psB[:], func=EXP,
                                         scale=0.125)
                    sts.append((stA, stB))
                attnv(nkt - 1, 0)
                attnv(nkt - 1, 1)

                # ---- evacuate + normalize ----
                # 1/den straight off the PSUM den row (reciprocal_approx_fast
                # is single-pass, ~1.2us on one lane), DRAM round trip only
                # for the partition broadcast; all DMAs contiguous.
                # den rows: PSUM -> SBUF block DMA ([32,32] per head,
                # 128B runs), one reciprocal, blocks back to DRAM in natural
                # q order for the partition_broadcast below
                uA = small.tile([128, 1024], fp32, tag="u", name=f"uA_{j}")
                nc.vector.tensor_copy(out=uA[0:DK + 1, :],
                                      in_=po0[0:DK + 1, :])
                nc.sync.dma_start(out=rds[2 * j:2 * j + 1, :],
                                  in_=uA[DK:DK + 1, :])
                uB = small.tile([128, 1024], fp32, tag="u", name=f"uB_{j}")
                nc.vector.tensor_copy(out=uB[0:DK + 1, :],
                                      in_=po1[0:DK + 1, :])
                nc.sync.dma_start(out=rds[2 * j + 1:2 * j + 2, :],
                                  in_=uB[DK:DK + 1, :])
                den2 = small.tile([32, 2, 32], fp32, tag="den",
                                  name=f"den_{j}")
                nc.gpsimd.dma_start(
                    out=den2[:],
                    in_=rds[2 * j:2 * j + 2, :]
                    .rearrange("h (p t) -> p h t", p=32))
                rec2 = small.tile([32, 2, 32], fp32, tag="rec",
                                  name=f"rec_{j}")
                nc.vector.reciprocal_approx_fast(out=rec2[:], in_=den2[:])
                nc.sync.dma_start(
                    out=rds2[2 * j:2 * j + 2, :]
                    .rearrange("h (p t) -> p h t", p=32),
                    in_=rec2[:])
                bcA = small.tile([128, 1024], fp32, tag="bc",
                                 name=f"bcA_{j}")
                nc.gpsimd.dma_start(
                    out=bcA[0:DK, :],
                    in_=rds2[2 * j:2 * j + 1, :].partition_broadcast(DK))
                nc.vector.tensor_mul(out=outTn[0:DK, j, :],
                                     in0=uA[0:DK, :], in1=bcA[0:DK, :])
                bcB = small.tile([128, 1024], fp32, tag="bc",
                                 name=f"bcB_{j}")
                nc.gpsimd.dma_start(
                    out=bcB[0:DK, :],
                    in_=rds2[2 * j + 1:2 * j + 2, :].partition_broadcast(DK))
                todd = small.tile([DK, 1024], bf16, tag="todd",
                                  name=f"todd_{j}")
                nc.vector.tensor_mul(out=todd[:],
                                     in0=uB[0:DK, :], in1=bcB[0:DK, :])
                nc.sync.dma_start(out=outTn[DK:128, j, :], in_=todd[:])

            # ---- output projection: one wave, 8 single-bank accumulators --
            P = [pst.tile([128, 1024], fp32, tag="st", name="pfA"),
                 pst.tile([128, 1024], fp32, tag="st", name="pfB"),
                 pout.tile([128, 1024], fp32, tag="po", name="pfC"),
                 pout.tile([128, 1024], fp32, tag="po", name="pfD")]
            for jj in range(4):
                for k8 in range(8):
                    pf = P[k8 // 2][:, (k8 % 2) * 512:(k8 % 2 + 1) * 512]
                    nc.tensor.matmul(
                        pf,
                        outTn[:, jj, k8 * 128:(k8 + 1) * 128],
                        wo_sb[:, jj, :],
                        start=(jj == 0), stop=(jj == 3))
            rings = [nc.sync, nc.scalar, nc.gpsimd]
            for k8 in range(8):
                pf = P[k8 // 2][:, (k8 % 2) * 512:(k8 % 2 + 1) * 512]
                ob = osb.tile([128, 512], fp32, tag="ob", name=f"ob_{k8}")
                nc.vector.tensor_add(out=ob[:], in0=pf, in1=bo2_sb[:])
                rings[k8 % 3].dma_start(
                    out=out[k8 * 128:(k8 + 1) * 128, :], in_=ob[:])

    nc.finalize()
    return nc


def _get_nc(skeys):
    if skeys not in _compiled:
        _compiled[skeys] = _build(skeys)
    return _compiled[skeys]


def kernel(query, key, value, key_padding_mask, Wq, bq, Wk, bk, Wv, bv,
           Wo, bo):
    global last_results
    from concourse.bass_utils import run_bass_kernel_spmd
    import ml_dtypes
    bf = ml_dtypes.bfloat16

    query = np.asarray(query, dtype=np.float32)
    key = np.asarray(key, dtype=np.float32)
    value = np.asarray(value, dtype=np.float32)
    mask = np.asarray(key_padding_mask).astype(bool)
    Wq = np.asarray(Wq, dtype=np.float32)
    Wk = np.asarray(Wk, dtype=np.float32)
    Wv = np.asarray(Wv, dtype=np.float32)
    Wo = np.asarray(Wo, dtype=np.float32)
    bqv = np.asarray(bq, dtype=np.float32)
    bvv = np.asarray(bv, dtype=np.float32)
    bov = np.asarray(bo, dtype=np.float32)

    # compact keys: keep only unmasked positions (zero-padded to SKC);
    # dense fallback when a batch keeps more than SKC. Masked/padded keys
    # carry zero v and a zero "ones" entry -> no mask bias needed anywhere.
    kept = [np.flatnonzero(~mask[b]) for b in range(B)]
    if max(len(k) for k in kept) <= SKC:
        skeys = SKC
        kc_l, vc_l, ko_l = [], [], []
        for b in range(B):
            n = len(kept[b])
            kc = np.zeros((skeys, D), np.float32)
            vc = np.zeros((skeys, D), np.float32)
            kc[:n] = key[b][kept[b]]
            vc[:n] = value[b][kept[b]]
            kones = np.zeros(skeys, np.float32)
            kones[:n] = 1.0
            kc_l.append(kc); vc_l.append(vc); ko_l.append(kones)
    else:
        skeys = S
        kc_l = [key[b] for b in range(B)]
        vc_l = [value[b] * (~mask[b])[:, None] for b in range(B)]
        ko_l = [(~mask[b]).astype(np.float32) for b in range(B)]

    nc = _get_nc(skeys)
    nkt = skeys // 128

    bo2 = bov + Wo @ bvv  # fold v bias through the output projection
    shared = {
        "wq": np.ascontiguousarray(Wq.T).astype(bf),
        "wk": np.ascontiguousarray(Wk.T).astype(bf),
        "wv": np.ascontiguousarray(Wv.T).astype(bf),
        "wo": np.ascontiguousarray(Wo.T).astype(bf),
        "bq": np.ascontiguousarray(bqv.reshape(4, 128).T),
        "bo2": bo2.reshape(1, D).astype(np.float32),
    }
    in_maps = []
    for c in range(N_CORES):
        b, qh = divmod(c, 2)
        qT = np.ascontiguousarray(query[b].T)
        m = {
            "xq": np.ascontiguousarray(
                qT[:, qh * SQ:(qh + 1) * SQ]).astype(bf),
            "xk": np.ascontiguousarray(kc_l[b].T).astype(bf),
            "xv": np.ascontiguousarray(vc_l[b].T).astype(bf),
            "ko": np.ascontiguousarray(
                ko_l[b].reshape(nkt, 128).T).astype(bf),
        }
        m.update(shared)
        in_maps.append(m)

    res = run_bass_kernel_spmd(nc, in_maps, list(range(N_CORES)))
    last_results = res

    out = np.empty((B, S, D), dtype=np.float32)
    for c in range(N_CORES):
        b, qh = divmod(c, 2)
        out[b, qh * SQ:(qh + 1) * SQ, :] = res.results[c]["out"]
    return out
